# revision 12
# baseline (speedup 1.0000x reference)
"""Trainium2 Bass kernel for nn_CrossAttention (sparse cross-attention).

Math (reference):
    q = xF @ Wq                    [B, L, H, D]
    k = context @ Wk               [B, K, H, D]
    v = context @ Wv               [B, K, H, D]
    attn = softmax(scale * q k^T)  over K
    out = (attn v) @ Wo + bo + xF, rows >= lengths[b] zeroed

Key algebraic restructure: context has only CONTEXT_CH=4 channels, so
k_h and v_h are rank-4.  Fold the projections:
    Wqk[ch, (h,e)] = scale * sum_d Wq[ch, h*D+d] * Wk[e, h*D+d]   [512, 32]
    s_h[row, key]  = (xF @ Wqk)_h[row, :] . context[key, :]
    Wvo_h[e, :]    = (Wv_h @ Wo_h)[e, :]                          [4, 512]
    out[row]       = sum_h (attn_h @ context) @ Wvo_h + bo + xF[row]
This collapses per-row FLOPs ~15x.  exp() stays elementwise (H*K per row)
and runs on the scalar engine; softmax max-subtraction is skipped (scores
are O(1), exp cannot overflow in fp32).

Sharding: only rows < lengths[b] are computed.  Valid rows are packed into
R-row supertiles (per batch, padded), distributed evenly across 8 cores.
Invalid rows are zero-filled on the host.  All device matmuls use fp32r.

Device dataflow is fully transposed (rows on the matmul free dim):
    xFT[ch, row] -> qkT = WqkS^T xFT -> sT_h[key, row] = ctxT^T qkT_h
    -> expT = exp(sT) -> avT'[(h,e)+sum, row] = ctx5^T expT
    -> normalize via PE-broadcast selector + reciprocal
    -> outT = WvoS^T avn; residual + bias fused into the PSUM drain on DVE
Head strips live at 32-partition granularity so score/av matmuls use
tensor-engine row/col tiling (4 concurrent 32-strip matmuls).

R=256 keeps the scores PSUM tile at 2 banks so it double-buffers
(scores of chunk kc+1 overlap exp of chunk kc on the scalar engine).
"""

import numpy as np

NUM_HEAD = 8
CH_HEAD = 64
CH = 512
CONTEXT_CH = 4
B, L, K = 16, 4096, 256
R = 256          # rows per supertile (matmul free dim)
N_CORES = 8


def _build_host_constants(Wq, Wk, Wv, Wo, bo):
    scale = CH_HEAD ** (-0.5)
    # Wqk[ch, 4h+e] = scale * sum_d Wq[ch, h*64+d] Wk[e, h*64+d]
    Wq_h = Wq.reshape(CH, NUM_HEAD, CH_HEAD)              # [512, 8, 64]
    Wk_h = Wk.reshape(CONTEXT_CH, NUM_HEAD, CH_HEAD)      # [4, 8, 64]
    Wqk = scale * np.einsum("chd,ehd->che", Wq_h, Wk_h)   # [512, 8, 4]

    # WqkS[pass, chunk, p, m]: lhsT for qkT matmul.  Output column m = 32g+e
    # holds head (4*pass+g), component e; other columns zero.
    WqkS = np.zeros((2, 4, 128, 128), np.float32)
    for p in range(2):
        for g in range(4):
            h = 4 * p + g
            for c in range(4):
                WqkS[p, c, :, 32 * g : 32 * g + CONTEXT_CH] = Wqk[
                    128 * c : 128 * (c + 1), h, :
                ]

    # WvoS[pass, 32g+e, :] = (Wv_h @ Wo_h)[e, :], row 32g+4 = bo/8 (the avn
    # row there is exactly 1.0 after normalization), rows 32g+5..31 = 0.
    Wv_h = Wv.reshape(CONTEXT_CH, NUM_HEAD, CH_HEAD)
    Wo_h = Wo.reshape(NUM_HEAD, CH_HEAD, CH)
    WvoS = np.zeros((2, 128, CH), np.float32)
    for p in range(2):
        for g in range(4):
            h = 4 * p + g
            WvoS[p, 32 * g : 32 * g + CONTEXT_CH, :] = Wv_h[:, h, :] @ Wo_h[h]
            WvoS[p, 32 * g + 4, :] = bo / NUM_HEAD
    WvoS = np.ascontiguousarray(WvoS)

    # Selector: Lam[m, row] = sum_k S[k, m] avT[k, row] = avT[32*(m//32)+4, row]
    Ssel = np.zeros((128, 128), np.float32)
    for g in range(4):
        Ssel[32 * g + 4, 32 * g : 32 * (g + 1)] = 1.0

    ident = np.eye(128, dtype=np.float32)
    return WqkS, WvoS, Ssel, ident


def _build_program(T):
    import concourse.bass as bass
    import concourse.tile as tile
    from concourse import bacc, mybir

    f32 = mybir.dt.float32
    f32r = mybir.dt.float32r
    f16 = mybir.dt.float16
    Exp = mybir.ActivationFunctionType.Exp

    nc = bacc.Bacc("TRN2", target_bir_lowering=False, debug=False)

    xft_d = nc.dram_tensor("xft", [T, 128, 4, R], f32r, kind="ExternalInput").ap()
    ctxt_d = nc.dram_tensor("ctxt", [T, 128, 256], f32r, kind="ExternalInput").ap()
    ctx5_d = nc.dram_tensor("ctx5", [T, 2, 128, 32], f16, kind="ExternalInput").ap()
    wqk_d = nc.dram_tensor("wqk", [2, 4, 128, 128], f32r, kind="ExternalInput").ap()
    wvo_d = nc.dram_tensor("wvo", [2, 128, CH], f32r, kind="ExternalInput").ap()
    ssel_d = nc.dram_tensor("ssel", [128, 128], f32r, kind="ExternalInput").ap()
    ident_d = nc.dram_tensor("ident", [128, 128], f32r, kind="ExternalInput").ap()
    out_d = nc.dram_tensor("outt", [T, 128, 4, R], f32, kind="ExternalOutput").ap()

    with tile.TileContext(nc) as tc:
        consts = tc.alloc_tile_pool(name="consts", bufs=1)
        wqk_s = consts.tile([128, 2, 4, 128], f32r)   # [p_ch, pass, chunk, m]
        wvo_s = consts.tile([128, 2, CH], f32r)
        ssel_s = consts.tile([128, 128], f32r)
        ident_s = consts.tile([128, 128], f32r)
        expbias = consts.tile([128, 1], f32)
        nc.vector.memset(expbias, -4.0)
        nc.sync.dma_start(out=wqk_s, in_=wqk_d.rearrange("a b p m -> p a b m"))
        nc.sync.dma_start(out=wvo_s, in_=wvo_d.rearrange("a p m -> p a m"))
        nc.sync.dma_start(out=ssel_s, in_=ssel_d)
        nc.sync.dma_start(out=ident_s, in_=ident_d)

        io = tc.alloc_tile_pool(name="io", bufs=3)
        ctxp = tc.alloc_tile_pool(name="ctxp", bufs=2)
        work = tc.alloc_tile_pool(name="work", bufs=2)
        expp = tc.alloc_tile_pool(name="expp", bufs=2)
        outp = tc.alloc_tile_pool(name="outp", bufs=2)

        # PSUM: sc 2x2 banks + qk 1 + av 1 + lam 1 + out 1 = 8 banks
        ps_qk = tc.alloc_tile_pool(name="ps_qk", bufs=1, space="PSUM")
        ps_sc = tc.alloc_tile_pool(name="ps_sc", bufs=1, space="PSUM")
        ps_av = tc.alloc_tile_pool(name="ps_av", bufs=1, space="PSUM")
        ps_out = tc.alloc_tile_pool(name="ps_out", bufs=1, space="PSUM")

        for t in range(T):
            xft = io.tile([128, 4, R], f32r)
            nc.sync.dma_start(out=xft, in_=xft_d[t])
            ctxt = ctxp.tile([128, 256], f32r)
            nc.sync.dma_start(out=ctxt, in_=ctxt_d[t])
            ctx5 = ctxp.tile([128, 2, 32], f16)
            nc.sync.dma_start(out=ctx5, in_=ctx5_d[t].rearrange("a p m -> p a m"))

            out_sb = outp.tile([128, 4, R], f32)
            avn = {}
            for p in range(2):
                # qkT[32g+e, row] for heads 4p+g
                qk_ps = ps_qk.tile([128, R], f32, tag="qk", padded_shape=[128, 512])
                for c in range(4):
                    nc.tensor.matmul(
                        out=qk_ps,
                        lhsT=wqk_s[:, p, c, :],
                        rhs=xft[:, c, :],
                        start=(c == 0),
                        stop=(c == 3),
                    )
                qk_sb = work.tile([128, R], f32r, tag="qk_sb")
                nc.vector.tensor_copy(out=qk_sb, in_=qk_ps)

                av_ps = ps_av.tile([128, R], f32, tag="av", padded_shape=[128, 512])
                # scores psum: one full bank per concurrent row-tiled group;
                # kc halves share the bank (start only on kc=0) so scores of
                # kc=1 overlap exp of kc=0.
                sc_ps = ps_sc.tile([128, 4, 512], f32, tag="sc")
                for kc in range(2):
                    for g in range(4):
                        nc.tensor.matmul(
                            out=sc_ps[:, g, R * kc : R * kc + R],
                            lhsT=ctxt[
                                32 * g : 32 * g + CONTEXT_CH,
                                128 * kc : 128 * (kc + 1),
                            ],
                            rhs=qk_sb[32 * g : 32 * g + CONTEXT_CH, :],
                            tile_position=(32 * g, 0),
                            start=(kc == 0),
                            stop=True,
                            skip_group_check=(kc == 1),
                        )
                    ex = expp.tile([128, 4, R], f16, tag="ex")
                    nc.scalar.activation(
                        out=ex, in_=sc_ps[:, :, R * kc : R * kc + R],
                        func=Exp, bias=expbias,
                    )
                    # av: 4 concurrent col-tiled matmuls (M=32 each),
                    # accumulating over the two key chunks
                    for g in range(4):
                        nc.tensor.matmul(
                            out=av_ps[32 * g : 32 * (g + 1), :],
                            lhsT=ctx5[:, kc, :],
                            rhs=ex[:, g, :],
                            tile_position=(0, 32 * g),
                            start=(kc == 0),
                            stop=(kc == 1),
                            skip_group_check=True,
                        )
                av_sb = work.tile([128, R], f32r, tag="av_sb")
                nc.vector.tensor_copy(out=av_sb, in_=av_ps)
                lam_ps = ps_av.tile([128, R], f32, tag="lam", padded_shape=[128, 512])
                nc.tensor.matmul(
                    out=lam_ps,
                    lhsT=ssel_s,
                    rhs=av_sb,
                )
                lr_sb = work.tile([128, R], f32, tag="lr_sb")
                nc.vector.reciprocal_approx_fast(out=lr_sb, in_=lam_ps)
                avn_sb = work.tile([128, R], f32r, tag=f"avn{p}")
                nc.vector.tensor_mul(avn_sb, av_sb, lr_sb)
                avn[p] = avn_sb

            for c in range(4):
                o_ps = ps_out.tile([128, R], f32, tag="o", padded_shape=[128, 512])
                nc.tensor.matmul(
                    out=o_ps,
                    lhsT=wvo_s[:, 0, 128 * c : 128 * (c + 1)],
                    rhs=avn[0],
                    start=True,
                    stop=False,
                )
                nc.tensor.matmul(
                    out=o_ps,
                    lhsT=wvo_s[:, 1, 128 * c : 128 * (c + 1)],
                    rhs=avn[1],
                    start=False,
                    stop=True,
                )
                # residual fused into the PSUM drain
                nc.vector.tensor_add(out_sb[:, c, :], o_ps, xft[:, c, :].bitcast(f32))

            nc.sync.dma_start(out=out_d[t], in_=out_sb)

        for pool in (ps_out, ps_av, ps_sc, ps_qk, outp, expp, work,
                     ctxp, io, consts):
            pool.release()

    nc.compile()
    return nc


def _plan_supertiles(lengths):
    """Split each batch's valid rows into R-row supertiles; spread over cores."""
    tiles = []  # (batch, row0, nvalid)
    for b in range(B):
        nb = int(lengths[b])
        r0 = 0
        while r0 < nb:
            tiles.append((b, r0, min(R, nb - r0)))
            r0 += R
    T = max(1, (len(tiles) + N_CORES - 1) // N_CORES)
    import os
    if os.environ.get("CAP_T"):
        T = min(T, int(os.environ["CAP_T"]))
    per_core = [tiles[c * T : (c + 1) * T] for c in range(N_CORES)]
    return per_core, T


def kernel(xF, context, lengths, Wq, Wk, Wv, Wo, bo):
    from concourse import bass_utils

    xF = np.asarray(xF, np.float32)
    context = np.asarray(context, np.float32)
    lengths_np = np.asarray(lengths, np.int32)

    WqkS, WvoS, Ssel, ident = _build_host_constants(
        np.asarray(Wq, np.float32),
        np.asarray(Wk, np.float32),
        np.asarray(Wv, np.float32),
        np.asarray(Wo, np.float32),
        np.asarray(bo, np.float32),
    )

    per_core, T = _plan_supertiles(lengths_np)
    nc = _build_program(T)

    # Per-batch context layouts
    ctxt_b = np.zeros((B, 128, 256), np.float32)
    ctx5_b = np.zeros((B, 2, 128, 32), np.float16)
    for b in range(B):
        cT = context[b].T  # [4, 256]
        for g in range(4):
            ctxt_b[b, 32 * g : 32 * g + CONTEXT_CH, :] = cT
        for kc in range(2):
            ctx5_b[b, kc, :, :CONTEXT_CH] = context[b, 128 * kc : 128 * (kc + 1), :]
            ctx5_b[b, kc, :, CONTEXT_CH] = 1.0

    in_maps = []
    for c in range(N_CORES):
        xft = np.zeros((T, 128, 4, R), np.float32)
        ctxt = np.zeros((T, 128, 256), np.float32)
        ctx5 = np.zeros((T, 2, 128, 32), np.float16)
        ctx5[:, :, :, CONTEXT_CH] = 1.0  # dummy tiles: finite normalizer
        for t, (b, r0, nv) in enumerate(per_core[c]):
            blockT = np.zeros((CH, R), np.float32)
            blockT[:, :nv] = xF[b, r0 : r0 + nv, :].T
            xft[t] = blockT.reshape(4, 128, R).transpose(1, 0, 2)
            ctxt[t] = ctxt_b[b]
            ctx5[t] = ctx5_b[b]
        in_maps.append(
            {
                "xft": xft,
                "ctxt": ctxt,
                "ctx5": ctx5,
                "wqk": WqkS,
                "wvo": WvoS,
                "ssel": Ssel,
                "ident": ident,
            }
        )

    import os

    trace = bool(os.environ.get("CA_TRACE"))
    res = bass_utils.run_bass_kernel_spmd(
        nc,
        in_maps,
        core_ids=list(range(N_CORES)),
        trace=trace,
        **({"tmpdir": "/tmp/ca_prof"} if trace else {}),
    )
    if trace and res.exec_time_ns is not None:
        print(f"HW exec time: {res.exec_time_ns} ns")

    out = np.zeros((B, L, CH), np.float32)
    for c in range(N_CORES):
        arr = res.results[c]["outt"]  # [T, 128, 4, R]
        for t, (b, r0, nv) in enumerate(per_core[c]):
            rows = arr[t].transpose(2, 1, 0).reshape(R, CH)  # [row, ch]
            out[b, r0 : r0 + nv, :] = rows[:nv]
    return out


# revision 13
# speedup vs baseline: 1.3162x; 1.3162x over previous
"""Trainium2 Bass kernel for nn_CrossAttention (sparse cross-attention).

Math (reference):
    q = xF @ Wq                    [B, L, H, D]
    k = context @ Wk               [B, K, H, D]
    v = context @ Wv               [B, K, H, D]
    attn = softmax(scale * q k^T)  over K
    out = (attn v) @ Wo + bo + xF, rows >= lengths[b] zeroed

Key algebraic restructure: context has only CONTEXT_CH=4 channels, so
k_h and v_h are rank-4.  Fold the projections:
    Wqk[ch, (h,e)] = scale * sum_d Wq[ch, h*D+d] * Wk[e, h*D+d]   [512, 32]
    s_h[row, key]  = (xF @ Wqk)_h[row, :] . context[key, :]
    Wvo_h[e, :]    = (Wv_h @ Wo_h)[e, :]                          [4, 512]
    out[row]       = sum_h (attn_h @ context) @ Wvo_h + bo + xF[row]
This collapses per-row FLOPs ~15x.  exp() stays elementwise (H*K per row)
and runs on the scalar engine; softmax max-subtraction is skipped (scores
are O(1), exp cannot overflow in fp32).

Sharding: only rows < lengths[b] are computed.  Valid rows are packed into
512-row supertiles (per batch, padded), distributed evenly across 8 cores.
Invalid rows are zero-filled on the host.  All device matmuls use fp32r.

Device dataflow is fully transposed (rows on the matmul free dim):
    xFT[ch, row] -> qkT = WqkS^T xFT -> sT_h[key, row] = ctxT^T qkT_h
    -> expT = exp(sT) -> avT'[(h,e)+sum, row] = ctx5^T expT
    -> normalize via PE-broadcast selector + reciprocal
    -> outT = WvoS^T avn; residual + bias fused into the PSUM drain on DVE
Head strips live at 32-partition granularity so score/av matmuls use
tensor-engine row/col tiling (4 concurrent 32-strip matmuls).

The tensor engine is kept busy through the serial scores->exp chain by
software pipelining: tile t's output matmuls are emitted inside tile t+1
(filling the exp(p0,kc0) wait), and pass 1's qk matmuls fill the
exp(p0,kc1) wait.  PSUM: scores 4 banks + qk 1 + av 1 + lam/out shared
pool 2 = 8 banks.
"""

import numpy as np

NUM_HEAD = 8
CH_HEAD = 64
CH = 512
CONTEXT_CH = 4
B, L, K = 16, 4096, 256
R = 512          # rows per supertile (matmul free dim)
N_CORES = 8


def _build_host_constants(Wq, Wk, Wv, Wo, bo):
    scale = CH_HEAD ** (-0.5)
    # Wqk[ch, 4h+e] = scale * sum_d Wq[ch, h*64+d] Wk[e, h*64+d]
    Wq_h = Wq.reshape(CH, NUM_HEAD, CH_HEAD)              # [512, 8, 64]
    Wk_h = Wk.reshape(CONTEXT_CH, NUM_HEAD, CH_HEAD)      # [4, 8, 64]
    Wqk = scale * np.einsum("chd,ehd->che", Wq_h, Wk_h)   # [512, 8, 4]

    # WqkS[pass, chunk, p, m]: lhsT for qkT matmul.  Output column m = 32g+e
    # holds head (4*pass+g), component e; other columns zero.
    WqkS = np.zeros((2, 4, 128, 128), np.float32)
    for p in range(2):
        for g in range(4):
            h = 4 * p + g
            for c in range(4):
                WqkS[p, c, :, 32 * g : 32 * g + CONTEXT_CH] = Wqk[
                    128 * c : 128 * (c + 1), h, :
                ]

    # WvoS[pass, 32g+e, :] = (Wv_h @ Wo_h)[e, :], row 32g+4 = bo/8 (the avn
    # row there is exactly 1.0 after normalization), rows 32g+5..31 = 0.
    Wv_h = Wv.reshape(CONTEXT_CH, NUM_HEAD, CH_HEAD)
    Wo_h = Wo.reshape(NUM_HEAD, CH_HEAD, CH)
    WvoS = np.zeros((2, 128, CH), np.float32)
    for p in range(2):
        for g in range(4):
            h = 4 * p + g
            WvoS[p, 32 * g : 32 * g + CONTEXT_CH, :] = Wv_h[:, h, :] @ Wo_h[h]
            WvoS[p, 32 * g + 4, :] = bo / NUM_HEAD
    WvoS = np.ascontiguousarray(WvoS)

    # Selector: Lam[m, row] = sum_k S[k, m] avT[k, row] = avT[32*(m//32)+4, row]
    Ssel = np.zeros((128, 128), np.float32)
    for g in range(4):
        Ssel[32 * g + 4, 32 * g : 32 * (g + 1)] = 1.0

    return WqkS, WvoS, Ssel


def _build_program(T):
    import concourse.bass as bass
    import concourse.tile as tile
    from concourse import bacc, mybir

    f32 = mybir.dt.float32
    f32r = mybir.dt.float32r
    f16 = mybir.dt.float16
    Exp = mybir.ActivationFunctionType.Exp

    nc = bacc.Bacc("TRN2", target_bir_lowering=False, debug=False)

    xft_d = nc.dram_tensor("xft", [T, 128, 4, R], f32r, kind="ExternalInput").ap()
    ctxt_d = nc.dram_tensor("ctxt", [T, 128, 256], f32r, kind="ExternalInput").ap()
    ctx5_d = nc.dram_tensor("ctx5", [T, 2, 128, 32], f16, kind="ExternalInput").ap()
    wqk_d = nc.dram_tensor("wqk", [2, 4, 128, 128], f32r, kind="ExternalInput").ap()
    wvo_d = nc.dram_tensor("wvo", [2, 128, CH], f32r, kind="ExternalInput").ap()
    ssel_d = nc.dram_tensor("ssel", [128, 128], f32r, kind="ExternalInput").ap()
    out_d = nc.dram_tensor("outt", [T, 128, 4, R], f32, kind="ExternalOutput").ap()

    with tile.TileContext(nc) as tc:
        consts = tc.alloc_tile_pool(name="consts", bufs=1)
        wqk_s = consts.tile([128, 2, 4, 128], f32r)   # [p_ch, pass, chunk, m]
        wvo_s = consts.tile([128, 2, CH], f32r)
        ssel_s = consts.tile([128, 128], f32r)
        expbias = consts.tile([128, 1], f32)
        nc.vector.memset(expbias, -4.0)
        nc.sync.dma_start(out=wqk_s, in_=wqk_d.rearrange("a b p m -> p a b m"))
        nc.sync.dma_start(out=wvo_s, in_=wvo_d.rearrange("a p m -> p a m"))
        nc.sync.dma_start(out=ssel_s, in_=ssel_d)

        io = tc.alloc_tile_pool(name="io", bufs=3)
        ctxp = tc.alloc_tile_pool(name="ctxp", bufs=2)
        work = tc.alloc_tile_pool(name="work", bufs=2)
        expp = tc.alloc_tile_pool(name="expp", bufs=3)
        outp = tc.alloc_tile_pool(name="outp", bufs=2)

        # PSUM budget (8 banks): sc 4 + qk 1 + av 1 + lam/out shared 2
        ps_qk = tc.alloc_tile_pool(name="ps_qk", bufs=1, space="PSUM")
        ps_sc = tc.alloc_tile_pool(name="ps_sc", bufs=1, space="PSUM")
        ps_av = tc.alloc_tile_pool(name="ps_av", bufs=1, space="PSUM")
        ps_lo = tc.alloc_tile_pool(name="ps_lo", bufs=2, space="PSUM")

        def emit_qk(xft, p):
            qk_ps = ps_qk.tile([128, R], f32, tag="qk", name=f"qk_ps{p}")
            for c in range(4):
                nc.tensor.matmul(
                    out=qk_ps,
                    lhsT=wqk_s[:, p, c, :],
                    rhs=xft[:, c, :],
                    start=(c == 0),
                    stop=(c == 3),
                )
            qk_sb = work.tile([128, R], f32r, tag="qk_sb", name=f"qk_sb{p}")
            nc.vector.tensor_copy(out=qk_sb, in_=qk_ps)
            return qk_sb

        def emit_scores(ctxt, qk_sb, kc):
            sc_ps = ps_sc.tile([128, 4, R], f32, tag="sc")
            for g in range(4):
                nc.tensor.matmul(
                    out=sc_ps[:, g, :],
                    lhsT=ctxt[
                        32 * g : 32 * g + CONTEXT_CH,
                        128 * kc : 128 * (kc + 1),
                    ],
                    rhs=qk_sb[32 * g : 32 * g + CONTEXT_CH, :],
                    tile_position=(32 * g, 0),
                )
            return sc_ps

        def emit_exp(sc_ps):
            ex = expp.tile([128, 4, R], f16, tag="ex")
            nc.scalar.activation(out=ex, in_=sc_ps, func=Exp, bias=expbias)
            return ex

        def emit_av(av_ps, ctx5, ex, kc):
            for g in range(4):
                nc.tensor.matmul(
                    out=av_ps[32 * g : 32 * (g + 1), :],
                    lhsT=ctx5[:, kc, :],
                    rhs=ex[:, g, :],
                    tile_position=(0, 32 * g),
                    start=(kc == 0),
                    stop=(kc == 1),
                    skip_group_check=True,
                )

        def emit_norm(av_ps, p):
            av_sb = work.tile([128, R], f32r, tag="av_sb", name=f"av_sb{p}")
            nc.vector.tensor_copy(out=av_sb, in_=av_ps)
            lam_ps = ps_lo.tile([128, R], f32, tag="lo", name=f"lam_ps{p}")
            nc.tensor.matmul(out=lam_ps, lhsT=ssel_s, rhs=av_sb)
            lr_sb = work.tile([128, R], f32, tag="lr_sb", name=f"lr_sb{p}")
            nc.vector.reciprocal_approx_fast(out=lr_sb, in_=lam_ps)
            avn_sb = work.tile([128, R], f32r, tag=f"avn{p}", name=f"avn_sb{p}")
            nc.vector.tensor_mul(avn_sb, av_sb, lr_sb)
            return avn_sb

        def emit_out(prev):
            avn, out_sb, xft, t = prev
            for c in range(4):
                o_ps = ps_lo.tile([128, R], f32, tag="lo", name=f"o_ps{c}")
                nc.tensor.matmul(
                    out=o_ps,
                    lhsT=wvo_s[:, 0, 128 * c : 128 * (c + 1)],
                    rhs=avn[0],
                    start=True,
                    stop=False,
                )
                nc.tensor.matmul(
                    out=o_ps,
                    lhsT=wvo_s[:, 1, 128 * c : 128 * (c + 1)],
                    rhs=avn[1],
                    start=False,
                    stop=True,
                )
                # residual + bias fused into the PSUM drain
                nc.vector.tensor_add(
                    out_sb[:, c, :], o_ps, xft[:, c, :].bitcast(f32)
                )
            nc.sync.dma_start(out=out_d[t], in_=out_sb)

        prev = None
        for t in range(T):
            xft = io.tile([128, 4, R], f32r)
            nc.sync.dma_start(out=xft, in_=xft_d[t])
            ctxt = ctxp.tile([128, 256], f32r)
            nc.sync.dma_start(out=ctxt, in_=ctxt_d[t])
            ctx5 = ctxp.tile([128, 2, 32], f16)
            nc.sync.dma_start(out=ctx5, in_=ctx5_d[t].rearrange("a p m -> p a m"))
            out_sb = outp.tile([128, 4, R], f32)
            avn = {}

            # ---- pass 0
            qk_sb0 = emit_qk(xft, 0)
            av_ps0 = ps_av.tile([128, R], f32, tag="av", name="av_ps0")
            sc00 = emit_scores(ctxt, qk_sb0, 0)
            # previous tile's output stage fills the exp(p0,kc0) wait
            if prev is not None:
                emit_out(prev)
            ex00 = emit_exp(sc00)
            sc01 = emit_scores(ctxt, qk_sb0, 1)
            emit_av(av_ps0, ctx5, ex00, 0)
            # pass 1 qk fills the exp(p0,kc1) wait
            qk_sb1 = emit_qk(xft, 1)
            ex01 = emit_exp(sc01)
            emit_av(av_ps0, ctx5, ex01, 1)
            avn[0] = emit_norm(av_ps0, 0)

            # ---- pass 1
            av_ps1 = ps_av.tile([128, R], f32, tag="av", name="av_ps1")
            sc10 = emit_scores(ctxt, qk_sb1, 0)
            ex10 = emit_exp(sc10)
            sc11 = emit_scores(ctxt, qk_sb1, 1)
            emit_av(av_ps1, ctx5, ex10, 0)
            ex11 = emit_exp(sc11)
            emit_av(av_ps1, ctx5, ex11, 1)
            avn[1] = emit_norm(av_ps1, 1)

            prev = (avn, out_sb, xft, t)

        emit_out(prev)

        for pool in (ps_lo, ps_av, ps_sc, ps_qk, outp, expp, work,
                     ctxp, io, consts):
            pool.release()

    nc.compile()
    return nc


def _plan_supertiles(lengths):
    """Split each batch's valid rows into R-row supertiles; spread over cores."""
    tiles = []  # (batch, row0, nvalid)
    for b in range(B):
        nb = int(lengths[b])
        r0 = 0
        while r0 < nb:
            tiles.append((b, r0, min(R, nb - r0)))
            r0 += R
    T = max(1, (len(tiles) + N_CORES - 1) // N_CORES)
    per_core = [tiles[c * T : (c + 1) * T] for c in range(N_CORES)]
    return per_core, T


def kernel(xF, context, lengths, Wq, Wk, Wv, Wo, bo):
    from concourse import bass_utils

    xF = np.asarray(xF, np.float32)
    context = np.asarray(context, np.float32)
    lengths_np = np.asarray(lengths, np.int32)

    WqkS, WvoS, Ssel = _build_host_constants(
        np.asarray(Wq, np.float32),
        np.asarray(Wk, np.float32),
        np.asarray(Wv, np.float32),
        np.asarray(Wo, np.float32),
        np.asarray(bo, np.float32),
    )

    per_core, T = _plan_supertiles(lengths_np)
    nc = _build_program(T)

    # Per-batch context layouts
    ctxt_b = np.zeros((B, 128, 256), np.float32)
    ctx5_b = np.zeros((B, 2, 128, 32), np.float16)
    for b in range(B):
        cT = context[b].T  # [4, 256]
        for g in range(4):
            ctxt_b[b, 32 * g : 32 * g + CONTEXT_CH, :] = cT
        for kc in range(2):
            ctx5_b[b, kc, :, :CONTEXT_CH] = context[b, 128 * kc : 128 * (kc + 1), :]
            ctx5_b[b, kc, :, CONTEXT_CH] = 1.0

    in_maps = []
    for c in range(N_CORES):
        xft = np.zeros((T, 128, 4, R), np.float32)
        ctxt = np.zeros((T, 128, 256), np.float32)
        ctx5 = np.zeros((T, 2, 128, 32), np.float16)
        ctx5[:, :, :, CONTEXT_CH] = 1.0  # dummy tiles: finite normalizer
        for t, (b, r0, nv) in enumerate(per_core[c]):
            blockT = np.zeros((CH, R), np.float32)
            blockT[:, :nv] = xF[b, r0 : r0 + nv, :].T
            xft[t] = blockT.reshape(4, 128, R).transpose(1, 0, 2)
            ctxt[t] = ctxt_b[b]
            ctx5[t] = ctx5_b[b]
        in_maps.append(
            {
                "xft": xft,
                "ctxt": ctxt,
                "ctx5": ctx5,
                "wqk": WqkS,
                "wvo": WvoS,
                "ssel": Ssel,
            }
        )

    import os

    trace = bool(os.environ.get("CA_TRACE"))
    res = bass_utils.run_bass_kernel_spmd(
        nc,
        in_maps,
        core_ids=list(range(N_CORES)),
        trace=trace,
        **({"tmpdir": "/tmp/ca_prof"} if trace else {}),
    )
    if trace and res.exec_time_ns is not None:
        print(f"HW exec time: {res.exec_time_ns} ns")

    out = np.zeros((B, L, CH), np.float32)
    for c in range(N_CORES):
        arr = res.results[c]["outt"]  # [T, 128, 4, R]
        for t, (b, r0, nv) in enumerate(per_core[c]):
            rows = arr[t].transpose(2, 1, 0).reshape(R, CH)  # [row, ch]
            out[b, r0 : r0 + nv, :] = rows[:nv]
    return out
